# revision 21
# baseline (speedup 1.0000x reference)
"""Local (sliding-window) causal attention kernel for Trainium2, 8 NeuronCores.

Problem: nn_LocalAttention (B=2, S=2048, D=1024, nh=16, hd=64, window=256,
topk=0).  q = x @ Wq.T ; k,v = reshaped inputs ; scores masked to the strict
causal band  qi-256 <= kj <= qi-1 ; softmax ; out = (P @ v) heads concat @ Wo.T.

Sharding: data-parallel over (B, S): 8 shards of 512 query rows; each core gets
its key/value halo of 768 rows.  No collectives.

v2 layout (everything feature-major, no on-device transposes):
  - qT = WqT.T @ xT (PE), evacuated into per-pair zero-padded qz tiles
    [128, 1024]: head h in rows 0:64 cols 0:512, head h+1 in rows 64:128 cols
    512:1024, rest zero.  The zeros make PAIRED score matmuls legal:
    one matmul with K=128 (both heads' features) and a block-strided moving
    AP computes both heads' score tiles at once (cross terms hit zeros).
  - score tiles: per pair, 4 PSUM tiles [128, 1024] f32 (2 banks), packed so
    every matmul output stays inside one 2KB PSUM bank:
      t0: [cj0 @0 w128 | cj1 @128 w256] (+pad), dup at +512 for head 2
      t1: [cj2 @0 w384] (+pad),          dup at +512
      t2: [cj3 @0 w384] (+pad),          dup at +512
      t3: [cj4 @0 w256 | cj5 @256 w128] (+pad), dup at +512
  - exp+mask: ONE activation + ONE mask multiply per tile over the
    [[512,2],[1,used]] AP (32 ACT instrs total vs 96 in v1).
  - attnT_unnorm[hd,qi], den[qi] = [v_h|1].T @ ss  (PE ones-column trick)
  - normalization: both dens of a pair parked at partitions 0/32 of a
    zeroed [33,512] tile, broadcast across 128 partitions by ONE K=33
    selector matmul (8 bc matmuls vs 16 in v1), then one DVE reciprocal
    + two multiplies into attnT.  (A DMA-broadcast variant measured 2x
    slower: HWDGE completion latency serialized the pair pipeline.)
  - software pipelining: per pair p the emission order is AV(p-1),
    scores(p), norm(p-1), so PE never waits on exp/mask of the current
    pair and the den->bc->recip->mul chain gets a full pair of slack.
  - big rep loads are double-buffered (pers2/wqx bufs=2) and emitted in
    consumption order (wq/x first) because HWDGE rings drain FIFO;
    va/wo/msk ride the ACT-engine ring, the rest the SP ring.
  - outT = WoT.T @ attnT (PE); host transposes back.

Matmul inputs bf16, accumulation fp32 PSUM.
"""

import os
import numpy as np

DTYPE = os.environ.get("LA_DTYPE", "bf16")
ATT_DB = os.environ.get("LA_ATTDB", "0") == "1"
OT_BUFS = int(os.environ.get("LA_OTBUFS", "4"))
RING_V7 = os.environ.get("LA_RINGV7", "0") == "1"
RING_V8 = os.environ.get("LA_RINGV8", "1") == "1"
RING_V9 = os.environ.get("LA_RINGV9", "1") == "1"
DEN_BUFS = int(os.environ.get("LA_DENBUFS", "4"))
RB_BUFS = int(os.environ.get("LA_RBBUFS", "3"))
RING_V10 = os.environ.get("LA_RINGV10", "0") == "1"  # kt on the now-idle ACT ring

NCORES = 8
B, S, D = 2, 2048, 1024
NH, HD = 16, 64
ROWS = 512            # query rows per core
HALO = 256            # window size
KROWS = ROWS + HALO   # 768 key rows per core
NKJ = KROWS // 128    # 6 key chunks

# qi-window of each kj-chunk cj: all qi chunks that the band of cj touches.
WIN = [(max(0, 128 * (cj - 2)), min(ROWS, 128 * cj + 128)) for cj in range(NKJ)]
WIDTHS = [hi - lo for lo, hi in WIN]

# Score-tile packing: 4 tiles per pair, each [128, 1024] (2 PSUM banks),
# head block stride 512.  Per tile: list of (cj, dst_off) with dst_off the
# column offset of chunk cj's window inside the head block.  Every chunk
# region [dst, dst+w) stays inside one 512-col (2KB) bank half, for both
# the +0 and +512 copies.
ST_TILES = [
    [(0, 0), (1, 128)],   # w128 @0..128, w256 @128..384  (bank-safe)
    [(2, 0)],             # w384 @0..384
    [(3, 0)],             # w384 @0..384
    [(4, 0), (5, 256)],   # w256 @0..256, w128 @256..384
]
ST_USED = [max(off + WIDTHS[cj] for cj, off in tl) for tl in ST_TILES]  # <=512
# chunk -> (tile idx, offset inside head block)
CHUNK_LOC = {cj: (ti, off) for ti, tl in enumerate(ST_TILES) for cj, off in tl}
MSLOT = [0, 1, 1, 2]  # tiles 1 and 2 have byte-identical masks: share slot
MSK_COLS = 3 * 1024   # mask tensor mirrors the distinct tile layouts

_prog = None  # cached compiled program


def _build_program(reps=1, phases=(1, 2, 3), hw_loop=False):
    from contextlib import ExitStack
    import concourse.tile as tile
    from concourse import bacc, mybir

    f32 = mybir.dt.float32
    DT = mybir.dt.bfloat16 if DTYPE == "bf16" else f32
    nc = bacc.Bacc("TRN2", target_bir_lowering=False, debug=False,
                   enable_asserts=False)

    # eps const AP (only 0.0/1.0 pre-registered) for the denominator guard.
    EPS = 1e-20
    eps_t = nc.alloc_sbuf_tensor("const-eps", [128, 1], f32)
    nc.gpsimd.memset(eps_t.ap(), EPS)
    nc.const_aps.aps[(f32, EPS)] = eps_t.ap()
    nc.all_engine_barrier()

    d_xT = nc.dram_tensor("xT", [D, ROWS], DT, kind="ExternalInput").ap()
    d_kT = nc.dram_tensor("kT", [D, KROWS], DT, kind="ExternalInput").ap()
    d_va = nc.dram_tensor("va", [KROWS, NH * 65], DT, kind="ExternalInput").ap()
    d_wq = nc.dram_tensor("wqT", [D, D], DT, kind="ExternalInput").ap()
    d_wo = nc.dram_tensor("woT", [D, D], DT, kind="ExternalInput").ap()
    d_msk = nc.dram_tensor("msk", [128, MSK_COLS], DT, kind="ExternalInput").ap()
    d_out = nc.dram_tensor("outT", [D, ROWS], DT, kind="ExternalOutput").ap()

    EXP = mybir.ActivationFunctionType.Exp

    def blk2(ap, used):
        """[128, 1024]-tile AP -> [[512,2],[1,used]] both-head view."""
        return ap.rearrange("p (b c) -> p b c", b=2)[:, :, 0:used]

    with tile.TileContext(nc) as tc, ExitStack() as ctx:
        pers = ctx.enter_context(tc.tile_pool(name="pers", bufs=1))
        pers2 = ctx.enter_context(tc.tile_pool(name="pers2", bufs=2))
        wqx = ctx.enter_context(tc.tile_pool(name="wqx", bufs=2))
        ps_mm = ctx.enter_context(tc.tile_pool(name="psmm", bufs=2, space="PSUM"))
        st_ps = ctx.enter_context(tc.tile_pool(name="stps", bufs=2, space="PSUM"))
        av_ps = ctx.enter_context(tc.tile_pool(name="avps", bufs=2, space="PSUM"))
        ss_pool = ctx.enter_context(tc.tile_pool(name="ssp", bufs=8))
        kt_pool = ctx.enter_context(tc.tile_pool(name="ktp", bufs=8 if (RING_V7 or RING_V8) else 3))
        den_pool = ctx.enter_context(tc.tile_pool(name="denp", bufs=4))
        r_pool = ctx.enter_context(tc.tile_pool(name="rp", bufs=4))
        rb_pool = ctx.enter_context(tc.tile_pool(name="rbp", bufs=RB_BUFS))
        ot_pool = ctx.enter_context(tc.tile_pool(name="otp", bufs=OT_BUFS))

        # qz: per-pair zero-padded q tiles, TWO sets so phase 1 of rep r+1
        # can be emitted (and run on PE) while rep r's attention still reads
        # the other set.  Zero blocks written once.
        qz_sets = [[pers.tile([128, 1024], DT, tag=f"qz{s}_{p}",
                              name=f"qz{s}_{p}") for p in range(8)]
                   for s in range(2)]
        for s in range(2):
            for p in range(8):
                nc.vector.memset(qz_sets[s][p][0:64, 512:1024], 0.0)
                nc.vector.memset(qz_sets[s][p][64:128, 0:512], 0.0)
        # bc selector: out row m gets den row 0 (m<64) or row 32 (m>=64).
        sel2 = pers.tile([33, 128], DT, tag="sel2", name="sel2")
        nc.vector.memset(sel2[:], 0.0)
        nc.vector.memset(sel2[0:1, 0:64], 1.0)
        nc.vector.memset(sel2[32:33, 64:128], 1.0)
        # den tiles: rows 1..31 must be zero (they enter the bc contraction).
        den_t = [pers.tile([33, ROWS], DT, tag=f"den{i}",
                           name=f"den{i}") for i in range(DEN_BUFS)]
        for i in range(DEN_BUFS):
            nc.vector.memset(den_t[i][:], 0.0)

        def emit_phase1(qz_t):
            # ---- phase 1: q projection into qz diagonal blocks ----
            # wq/x DMAs are emitted FIRST: the SP HWDGE ring is FIFO, so
            # queueing the 7MB of va/msk/wo (needed only later) ahead of
            # them would stall qproj behind ~16us of transfers.
            wq_t, x_t = [], []
            for k2 in range(8):
                t = wqx.tile([128, D], DT, tag=f"wq{k2}", name=f"wq{k2}")
                nc.sync.dma_start(out=t[:], in_=d_wq[128 * k2:128 * k2 + 128, :])
                wq_t.append(t)
            for k2 in range(8):
                t = wqx.tile([128, ROWS], DT, tag=f"x{k2}", name=f"x{k2}")
                nc.sync.dma_start(out=t[:], in_=d_xT[128 * k2:128 * k2 + 128, :])
                x_t.append(t)
            for m in range(8):
                if 1 in phases:
                    ps = ps_mm.tile([128, ROWS], f32, tag="mm", name="mm_ps_t")
                    for k2 in range(8):
                        nc.tensor.matmul(ps[:],
                                         wq_t[k2][:, 128 * m:128 * m + 128],
                                         x_t[k2][:], start=(k2 == 0),
                                         stop=(k2 == 7))
                    nc.scalar.copy(out=qz_t[m][0:64, 0:512], in_=ps[0:64, :])
                    nc.vector.tensor_copy(qz_t[m][64:128, 512:1024],
                                          ps[64:128, :])
                else:
                    nc.vector.memset(qz_t[m][0:64, 0:512], 0.01)
                    nc.vector.memset(qz_t[m][64:128, 512:1024], 0.01)

        def _rep_body(rep):
            ri = rep % 2 if rep is not None else 0
            qz_t = qz_sets[ri]
            sfx = f"{ri}_" if ATT_DB else ""
            attnT = [pers.tile([128, ROWS], DT, tag=f"at{sfx}{p}",
                               name=f"at{sfx}{p}") for p in range(8)]

            # ---- bulk loads for attention / output projection ----
            va_t = []
            for cj in range(NKJ):
                t = pers2.tile([128, NH * 65], DT, tag=f"va{cj}", name=f"va{cj}")
                (nc.gpsimd if RING_V9 else nc.scalar).dma_start(out=t[:], in_=d_va[128 * cj:128 * cj + 128, :])
                va_t.append(t)
            msk_t = pers2.tile([128, MSK_COLS], DT, tag="msk")
            (nc.gpsimd if RING_V9 else nc.scalar).dma_start(out=msk_t[:], in_=d_msk[:, :])
            wo_t = []
            for t2 in range(8):
                t = pers2.tile([128, D], DT, tag=f"wo{t2}", name=f"wo{t2}")
                weng = nc.sync if RING_V7 else (
                    nc.gpsimd if RING_V9 else nc.scalar)
                weng.dma_start(out=t[:], in_=d_wo[128 * t2:128 * t2 + 128, :])
                wo_t.append(t)

            # ---- phase 2: attention, software-pipelined over pairs ----
            kt_pre = {}
            if RING_V7 or RING_V8:
                for p in range(8):
                    t = kt_pool.tile([128, KROWS], DT, tag="kt", name="kt_p")
                    (nc.scalar if RING_V10 else nc.sync).dma_start(
                        out=t[:], in_=d_kT[128 * p:128 * p + 128, :])
                    kt_pre[p] = t

            def emit_scores(p, st_ps):
                if RING_V7 or RING_V8:
                    kt = kt_pre[p]
                else:
                    kt = kt_pool.tile([128, KROWS], DT, tag="kt", name="kt_p")
                    nc.sync.dma_start(out=kt[:], in_=d_kT[128 * p:128 * p + 128, :])
                qz3 = qz_t[p][:].rearrange("p (b c) -> p b c", b=2)
                ss_tiles = []
                for ti, tl in enumerate(ST_TILES):
                    used = ST_USED[ti]
                    sp = st_ps.tile([128, 1024], f32, tag="st", name="sp_st")
                    sp3 = sp[:].rearrange("p (b c) -> p b c", b=2)
                    for cj, off in tl:
                        lo, hi = WIN[cj]
                        w = hi - lo
                        for bb in range(2):
                            nc.tensor.matmul(
                                sp3[:, bb:bb + 1, off:off + w],
                                kt[:, 128 * cj:128 * cj + 128],
                                qz3[:, bb:bb + 1, lo:hi],
                                start=True, stop=True, skip_group_check=True)
                    ss = ss_pool.tile([128, 1024], DT, tag="ss", name="ss_st")
                    nc.scalar.activation(blk2(ss[:], used), blk2(sp[:], used),
                                         EXP, scale=0.125)
                    mo = 1024 * MSLOT[ti]
                    nc.vector.tensor_mul(
                        blk2(ss[:], used), blk2(ss[:], used),
                        blk2(msk_t[:, mo:mo + 1024], used))
                    ss_tiles.append(ss)
                return ss_tiles

            def emit_av_mm(p, ss_tiles, av_ps):
                """AV matmuls + den extraction + reciprocal + rb broadcast."""
                av_pair = []
                for sub in range(2):
                    h = 2 * p + sub
                    av = av_ps.tile([65, ROWS], f32, tag="av", name="av_ps")
                    for cj in range(NKJ):
                        lo, hi = WIN[cj]
                        w = hi - lo
                        ti, off = CHUNK_LOC[cj]
                        src = ss_tiles[ti][:, 512 * sub + off:512 * sub + off + w]
                        nc.tensor.matmul(
                            av[:, lo:hi],
                            va_t[cj][:, 65 * h:65 * h + 65],
                            src,
                            start=(cj == 0), stop=(cj == NKJ - 1),
                            skip_group_check=True)
                    av_pair.append(av)
                # dens parked on partitions 0 and 32 (engine APs must start
                # on a quadrant boundary); rows 1..31 stay zero.
                den2 = den_t[p % DEN_BUFS]
                # one den extract on ACT, one on DVE: balances the two
                # PSUM-evacuation engines (ACT is the attention rate-setter)
                nc.scalar.add(den2[0:1, :], av_pair[0][64:65, :], EPS)
                nc.vector.tensor_scalar_add(den2[32:33, :],
                                            av_pair[1][64:65, :], EPS)
                return av_pair, den2

            def emit_norm(p, av_pair, den2):
                bc = ps_mm.tile([128, ROWS], f32, tag="mm", name="mm_ps_t")
                nc.tensor.matmul(bc[:], sel2[:], den2[:],
                                 start=True, stop=True)
                rb = rb_pool.tile([128, ROWS], f32, tag="rb", name="rb_p")
                nc.vector.reciprocal_approx_fast(out=rb[:], in_=bc[:])
                for sub in range(2):
                    nc.vector.tensor_mul(
                        attnT[p][64 * sub:64 * sub + 64, :],
                        av_pair[sub][0:64, :],
                        rb[64 * sub:64 * sub + 64, :])

            if 2 in phases:
                prev = None
                for p in range(8):
                    if prev is not None:
                        prev_av = emit_av_mm(prev[0], prev[1], av_ps)
                    ss_tiles = emit_scores(p, st_ps)
                    if prev is not None:
                        emit_norm(prev[0], *prev_av)
                    prev = (p, ss_tiles)
                prev_av = emit_av_mm(prev[0], prev[1], av_ps)
                emit_norm(prev[0], *prev_av)
            else:
                for p2x in range(8):
                    nc.vector.memset(attnT[p2x][:], 0.01)

            # ---- phase 1 of the NEXT rep: emitted before phase 3 so the
            # PE fills its attention-phase idle with projection work and
            # the next rep's scores are unblocked as early as possible.
            if rep is not None:
                emit_phase1(qz_sets[(rep + 1) % 2])

            # ---- phase 3: output projection ----
            if 3 in phases:
                for n in range(8):
                    ps = ps_mm.tile([128, ROWS], f32, tag="mm", name="mm_ps_t")
                    for t2 in range(8):
                        nc.tensor.matmul(ps[:],
                                         wo_t[t2][:, 128 * n:128 * n + 128],
                                         attnT[t2][:], start=(t2 == 0),
                                         stop=(t2 == 7))
                    ot = ot_pool.tile([128, ROWS], DT, tag="ot", name="ot_sb")
                    if n % 2 == 0:
                        nc.vector.tensor_copy(ot[:], ps[:])
                    else:
                        nc.scalar.copy(out=ot[:], in_=ps[:])
                    eng = (nc.scalar if RING_V7 else
                           (nc.gpsimd if RING_V8 else nc.sync))
                    eng.dma_start(
                        out=d_out[128 * n:128 * n + 128, :], in_=ot[:])

        if hw_loop:
            # uniform loop body: no cross-rep pipelining (measurement only)
            emit_phase1(qz_sets[0])
            with tc.For_i(0, hw_loop):
                for _rep in range(reps):
                    _rep_body(None)
                    emit_phase1(qz_sets[0])
        else:
            emit_phase1(qz_sets[0])
            for _rep in range(reps - 1):
                _rep_body(_rep)
            # last rep: no next-rep prefetch
            qz_sets[reps % 2] = qz_sets[(reps - 1) % 2]
            _rep_body(None)

    nc.compile()
    return nc


def _to_dt(a):
    if DTYPE == "bf16":
        import ml_dtypes
        return np.ascontiguousarray(a).astype(ml_dtypes.bfloat16)
    return np.ascontiguousarray(a).astype(np.float32)


def _host_prep(query_seq, keys_seq, values_seq, Wq, Wo):
    """Build the 8 per-core input maps."""
    qT_all = np.ascontiguousarray(query_seq.transpose(0, 2, 1))  # [B, D, S]
    kT_all = np.ascontiguousarray(keys_seq.transpose(0, 2, 1))
    wqT = _to_dt(Wq.T)
    woT = _to_dt(Wo.T)

    def band_mask(first):
        m = np.zeros((128, MSK_COLS), np.float32)
        for cj in range(NKJ):
            lo, hi = WIN[cj]
            w = hi - lo
            ti, off = CHUNK_LOC[cj]
            ti = MSLOT[ti]
            kj = 128 * cj + np.arange(128)[:, None]
            qi = np.arange(lo, hi)[None, :]
            valid = (kj >= qi) & (kj <= qi + HALO - 1)
            if first:
                valid &= (kj >= HALO)
            blk = valid.astype(np.float32)
            for sub in range(2):  # same mask for both heads of the pair
                c0 = 1024 * ti + 512 * sub + off
                m[:, c0:c0 + w] = blk
        return m

    msk_first = _to_dt(band_mask(True))
    msk_rest = _to_dt(band_mask(False))

    in_maps = []
    for c in range(NCORES):
        b, ch = c // 4, c % 4
        r0 = ch * ROWS
        xT = _to_dt(qT_all[b][:, r0:r0 + ROWS])
        kT = np.zeros((D, KROWS), np.float32)
        va = np.zeros((KROWS, NH * 65), np.float32)
        va[:, 64::65] = 1.0  # ones column per head
        if ch == 0:
            kT[:, HALO:] = kT_all[b][:, 0:ROWS]
            v_halo = values_seq[b, 0:ROWS]
            va[HALO:, :] = np.concatenate(
                [v_halo.reshape(ROWS, NH, HD),
                 np.ones((ROWS, NH, 1), np.float32)], axis=2).reshape(ROWS, -1)
        else:
            kT[:, :] = kT_all[b][:, r0 - HALO:r0 + ROWS]
            v_halo = values_seq[b, r0 - HALO:r0 + ROWS]
            va[:, :] = np.concatenate(
                [v_halo.reshape(KROWS, NH, HD),
                 np.ones((KROWS, NH, 1), np.float32)], axis=2).reshape(KROWS, -1)
        in_maps.append({
            "xT": xT, "kT": _to_dt(kT), "va": _to_dt(va), "wqT": wqT,
            "woT": woT, "msk": msk_first if ch == 0 else msk_rest,
        })
    return in_maps


def _run(inputs, trace=False):
    global _prog
    from concourse.bass_utils import run_bass_kernel_spmd

    query_seq = np.asarray(inputs["query_seq"], np.float32)
    keys_seq = np.asarray(inputs["keys_seq"], np.float32)
    values_seq = np.asarray(inputs["values_seq"], np.float32)
    Wq = np.asarray(inputs["Wq"], np.float32)
    Wo = np.asarray(inputs["Wo"], np.float32)
    assert int(inputs.get("window", HALO)) == HALO
    assert int(inputs.get("topk", 0)) == 0

    if _prog is None:
        _prog = _build_program()

    in_maps = _host_prep(query_seq, keys_seq, values_seq, Wq, Wo)
    res = run_bass_kernel_spmd(_prog, in_maps, list(range(NCORES)), trace=trace)

    out = np.empty((B, S, D), np.float32)
    for c in range(NCORES):
        b, ch = c // 4, c % 4
        r0 = ch * ROWS
        out[b, r0:r0 + ROWS, :] = res.results[c]["outT"].T.astype(np.float32)
    return out, res


def kernel(**inputs):
    out, _ = _run(inputs)
    if np.isnan(out).any():  # rare transient first-dispatch flake: retry once
        out, _ = _run(inputs)
    return out

